# revision 7
# baseline (speedup 1.0000x reference)
"""Trainium2 Bass kernel for nn_BaseInformationLoss.

Computes, from activity (1024, 64) f32 and k (=5):
  out[0]  = kNN (Kozachenko-Leonenko) joint entropy in bits
  out[1:] = 64 KDE marginal entropies (reflected Gaussian KDE)

Sharding: 8 cores, each handles 128 of the 1024 query rows (data-parallel
over rows of the pairwise tensors); every core holds the full activity for
the sample side. Per-core partial sums (sum of log rho_sq over its rows and
sum of log density over its rows per feature) are combined on the host.

Device-side structure per core:
  - AT (64, 1024) = activity^T via PE transposes; AT2 = AT^2.
  - kNN: D_neg = 2*dot - ||a_j||^2 via PE matmul; top-8 per row via DVE max;
    rho_sq = ||a_i||^2 - D_neg_top[k]; log via ACT; partition-sum via PE.
  - KDE: for each feature r and mirror m, (x_i - y_j)^2 computed as a K=3
    PE matmul (x^2*1 + (-2x)*y + 1*y^2); ACT Exp with per-feature scale
    -1/(2h^2) and accum_out produces the j-sum directly.
"""

import math

import numpy as np

B = 1024
N = 64
P = 128
NCORES = 8
RB = 32  # features per RHS partition-block (3*RB = 96 partitions)

_BUILD_CACHE = {}


def _build(k: int):
    import concourse.bass as bass
    import concourse.mybir as mybir
    from concourse import tile

    f32 = mybir.dt.float32
    AF = mybir.ActivationFunctionType
    OP = mybir.AluOpType

    nc = bass.Bass()
    act_full = nc.dram_tensor("act_full", [B, N], f32, kind="ExternalInput")
    act_rows = nc.dram_tensor("act_rows", [P, N], f32, kind="ExternalInput")
    ident_d = nc.dram_tensor("ident", [P, P], f32, kind="ExternalInput")
    out_d = nc.dram_tensor("partials", [N + 1], f32, kind="ExternalOutput")

    c_bw = 1.06 * B ** (-0.2)
    c_den = 1.0 / (B * math.sqrt(2.0 * math.pi))

    with tile.TileContext(nc) as tc:
        with tc.tile_pool(name="sb", bufs=1) as sb:
            # activation() converts float biases via nc.const_aps; register
            # the values we use as memset tiles.
            for cval in (0.0, -2.0, 1e-8):
                cb = sb.tile([P, 1], f32, tag=f"const{cval}")
                nc.vector.memset(cb[:], cval)
                nc.const_aps.aps[(f32, cval)] = cb[:]

            a1 = sb.tile([P, B // P, N], f32)
            arows = sb.tile([P, N], f32)
            id128 = sb.tile([P, P], f32)
            AT = sb.tile([N, B], f32)
            AT2 = sb.tile([N, B], f32)
            ATsl = sb.tile([N, P], f32)
            ATsl2 = sb.tile([N, P], f32)
            nc.sync.dma_start(a1[:], act_full.rearrange("(n p) d -> p n d", p=P))
            nc.sync.dma_start(arows[:], act_rows[:])
            nc.sync.dma_start(id128[:], ident_d[:])

            with tc.tile_pool(name="ps_t", bufs=2, space="PSUM") as ps_t:
                for c in range(B // P):
                    pst = ps_t.tile([N, P], f32, tag="tp")
                    nc.tensor.transpose(pst[:], a1[:, c, :], id128[:])
                    nc.vector.tensor_copy(AT[:, c * P:(c + 1) * P], pst[:])
                pst = ps_t.tile([N, P], f32, tag="tp")
                nc.tensor.transpose(pst[:], arows[:], id128[:])
                nc.vector.tensor_copy(ATsl[:], pst[:])

            ss = sb.tile([N, 1], f32)
            sums = sb.tile([N, 1], f32)
            nc.scalar.activation(AT2[:], AT[:], AF.Square, accum_out=ss[:])
            nc.vector.tensor_reduce(sums[:], AT[:], mybir.AxisListType.X, OP.add)
            nc.scalar.activation(ATsl2[:], ATsl[:], AF.Square)

            # ---- bandwidth stats: h, exp scale, density scale (all (64,1)) ----
            mean = sb.tile([N, 1], f32)
            m2 = sb.tile([N, 1], f32)
            t2 = sb.tile([N, 1], f32)
            t3 = sb.tile([N, 1], f32)
            var = sb.tile([N, 1], f32)
            std = sb.tile([N, 1], f32)
            h = sb.tile([N, 1], f32)
            hinv = sb.tile([N, 1], f32)
            cden = sb.tile([N, 1], f32)
            h2 = sb.tile([N, 1], f32)
            h2inv = sb.tile([N, 1], f32)
            escale = sb.tile([N, 1], f32)
            nc.vector.tensor_scalar(mean[:], sums[:], 1.0 / B, None, OP.mult)
            nc.vector.tensor_tensor(m2[:], mean[:], mean[:], OP.mult)
            nc.vector.tensor_scalar(t2[:], ss[:], 1.0 / (B - 1), None, OP.mult)
            nc.vector.tensor_scalar(t3[:], m2[:], B / (B - 1.0), None, OP.mult)
            nc.vector.tensor_tensor(var[:], t2[:], t3[:], OP.subtract)
            nc.scalar.activation(std[:], var[:], AF.Sqrt)
            nc.vector.tensor_scalar(h[:], std[:], c_bw, 1e-4, OP.mult, OP.max)
            nc.vector.reciprocal(hinv[:], h[:])
            nc.vector.tensor_scalar(cden[:], hinv[:], c_den, None, OP.mult)
            nc.vector.tensor_tensor(h2[:], h[:], h[:], OP.mult)
            nc.vector.reciprocal(h2inv[:], h2[:])
            nc.vector.tensor_scalar(escale[:], h2inv[:], -0.5, None, OP.mult)

            # transpose (64,1) -> (1,64) rows via tiny SBUF->SBUF DMAs
            erow = sb.tile([1, N], f32)
            crow = sb.tile([1, N], f32)
            nc.sync.dma_start(erow[:], escale[:])
            nc.sync.dma_start(crow[:], cden[:])

            ones1c = sb.tile([1, P], f32)
            ones128c = sb.tile([P, 1], f32)
            negones = sb.tile([N, 1], f32)
            nc.vector.memset(ones1c[:], 1.0)
            nc.vector.memset(ones128c[:], 1.0)
            nc.vector.memset(negones[:], -1.0)

            # broadcast escale/cden rows across 128 partitions: ones^T @ row
            Eb = sb.tile([P, N], f32)
            Cb = sb.tile([P, N], f32)
            with tc.tile_pool(name="ps_b", bufs=2, space="PSUM") as ps_b:
                psb = ps_b.tile([P, N], f32, tag="bc")
                nc.tensor.matmul(psb[:], ones1c[:], erow[:])
                nc.vector.tensor_copy(Eb[:], psb[:])
                psb = ps_b.tile([P, N], f32, tag="bc")
                nc.tensor.matmul(psb[:], ones1c[:], crow[:])
                nc.vector.tensor_copy(Cb[:], psb[:])

            # ---- kNN part ----
            lhs2 = sb.tile([N, P], f32)  # 2 * x_i rows
            nc.vector.tensor_scalar(lhs2[:], ATsl[:], 2.0, None, OP.mult)
            negsq = sb.tile([1, B], f32)  # -||a_j||^2
            with tc.tile_pool(name="ps_n", bufs=2, space="PSUM") as ps_n:
                for hh in range(2):
                    psn = ps_n.tile([1, B // 2], f32, tag="nn")
                    nc.tensor.matmul(psn[:], negones[:], AT2[:, hh * 512:(hh + 1) * 512])
                    nc.vector.tensor_copy(negsq[:, hh * 512:(hh + 1) * 512], psn[:])

            sq_i = sb.tile([P, 1], f32)
            sqsc = sb.tile([P, N], f32)
            nc.scalar.activation(sqsc[:], arows[:], AF.Square, accum_out=sq_i[:])

            max8t = sb.tile([P, 8], f32)
            rho = sb.tile([P, 1], f32)
            rhoc = sb.tile([P, 1], f32)
            lnrho = sb.tile([P, 1], f32)
            joint_sb = sb.tile([1, 1], f32)
            with tc.tile_pool(name="ps_d", bufs=1, space="PSUM") as ps_d:
                psD = ps_d.tile([P, B], f32, tag="dist")
                for hh in range(2):
                    sl = slice(hh * 512, (hh + 1) * 512)
                    nc.tensor.matmul(psD[:, sl], lhs2[:], AT[:, sl], start=True, stop=False)
                    nc.tensor.matmul(psD[:, sl], ones1c[:], negsq[:, sl], start=False, stop=True)
                nc.vector.max(max8t[:], psD[:])
            nc.vector.tensor_scalar(rho[:], max8t[:, k:k + 1], -1.0, sq_i[:], OP.mult, OP.add)
            nc.vector.tensor_scalar(rhoc[:], rho[:], 1e-12, None, OP.max)
            nc.scalar.activation(lnrho[:], rhoc[:], AF.Ln)
            with tc.tile_pool(name="ps_j", bufs=1, space="PSUM") as ps_j:
                psj = ps_j.tile([1, 1], f32, tag="j")
                nc.tensor.matmul(psj[:], lnrho[:], ones128c[:])
                nc.vector.tensor_copy(joint_sb[:], psj[:])
            nc.sync.dma_start(out_d[0:1], joint_sb[:])

            # ---- KDE lhs staging ----
            # (xi - y)^2    = xi^2 + (-2xi)*y + 1*y^2
            # (xi + y)^2    = xi^2 + (+2xi)*y + 1*y^2
            # (xi - 2 + y)^2 = zi^2 + (2zi)*y + 1*y^2, zi = xi - 2
            # The y/y^2 terms come from a K=2 PE matmul; the xi^2/zi^2 term is
            # folded into the ACT Exp bias (per-partition = per-query).
            # lhsT stack: partition 0 = linear coef per (mirror, r); p1 = ones.
            m2xi = sb.tile([N, P], f32)
            z2xi = sb.tile([N, P], f32)
            nc.vector.tensor_scalar(m2xi[:], ATsl[:], -2.0, None, OP.mult)
            nc.vector.tensor_scalar(z2xi[:], ATsl[:], 2.0, -4.0, OP.mult, OP.add)

            # partition 0 = ones (y^2 coef), partition 1 = linear coef
            # (engine memset must start at partition 0; DMA can hit p1)
            LHS = sb.tile([2, 3 * N * P], f32)
            seg = N * P  # 8192
            nc.vector.memset(LHS[0:1, :], 1.0)
            nc.sync.dma_start(LHS[1:2, 0 * seg:1 * seg], m2xi[:])
            nc.sync.dma_start(LHS[1:2, 1 * seg:2 * seg], lhs2[:])
            nc.sync.dma_start(LHS[1:2, 2 * seg:3 * seg], z2xi[:])

            # Exp biases: escale_r * xi^2 (mirrors 0/1), escale_r * zi^2 (m 2)
            zsq_q = sb.tile([P, N], f32)
            Bias01 = sb.tile([P, N], f32)
            Bias2q = sb.tile([P, N], f32)
            nc.scalar.activation(zsq_q[:], arows[:], AF.Square, bias=-2.0)
            nc.vector.tensor_tensor(Bias01[:], sqsc[:], Eb[:], OP.mult)
            nc.vector.tensor_tensor(Bias2q[:], zsq_q[:], Eb[:], OP.mult)

            # ---- KDE main loop ----
            S6 = sb.tile([P, N, 3], f32)
            expout = sb.tile([P, B], f32)
            with (
                tc.tile_pool(name="ps_m", bufs=4, space="PSUM") as ps_m,
                tc.tile_pool(name="rhs_p", bufs=6) as rhs_p,
            ):
                for r in range(N):
                    rhs_t = rhs_p.tile([2, B], f32, tag="rhs")
                    nc.sync.dma_start(rhs_t[0:1, :], AT2[r:r + 1, :])
                    nc.sync.dma_start(rhs_t[1:2, :], AT[r:r + 1, :])
                    for m in range(3):
                        lhs_ap = LHS[:, (m * N + r) * P:(m * N + r + 1) * P]
                        bias_ap = (Bias2q if m == 2 else Bias01)[:, r:r + 1]
                        pst2 = ps_m.tile([P, B], f32, tag="t2")
                        for hh in range(2):
                            sl = slice(hh * 512, (hh + 1) * 512)
                            nc.tensor.matmul(pst2[:, sl], lhs_ap, rhs_t[:, sl],
                                             start=True, stop=True)
                        nc.scalar.activation(expout[:], pst2[:], AF.Exp,
                                             scale=Eb[:, r:r + 1],
                                             bias=bias_ap,
                                             accum_out=S6[:, r, m:m + 1])

            # ---- finalize marginals ----
            S = sb.tile([P, N], f32)
            Smul = sb.tile([P, N], f32)
            lnS = sb.tile([P, N], f32)
            margp = sb.tile([N, 1], f32)
            nc.vector.tensor_reduce(S[:], S6[:], mybir.AxisListType.X, OP.add)
            nc.vector.tensor_tensor(Smul[:], S[:], Cb[:], OP.mult)
            nc.scalar.activation(lnS[:], Smul[:], AF.Ln, bias=1e-8)
            with tc.tile_pool(name="ps_f", bufs=1, space="PSUM") as ps_f:
                psf = ps_f.tile([N, 1], f32, tag="f")
                nc.tensor.matmul(psf[:], lnS[:], ones128c[:])
                nc.vector.tensor_copy(margp[:], psf[:])
            nc.sync.dma_start(out_d[1:N + 1], margp[:])

    # Walrus codegen rejects wait+update sync directly on Activation-queue
    # instructions ("Too many sync wait commands"); split sync onto
    # standalone event-semaphore instructions like bacc does.
    bass._bass_rust.generate_event_semaphores(nc)
    return nc


def _get_nc(k: int):
    if k not in _BUILD_CACHE:
        _BUILD_CACHE[k] = _build(k)
    return _BUILD_CACHE[k]


def kernel(activity, k=5):
    from concourse.bass_utils import run_bass_kernel_spmd

    A = np.ascontiguousarray(np.asarray(activity, dtype=np.float32))
    assert A.shape == (B, N), A.shape
    kk = int(k)
    assert 0 <= kk <= 6, f"top-8 path requires k <= 6, got {kk}"

    nc = _get_nc(kk)
    ident = np.eye(P, dtype=np.float32)
    in_maps = [
        {"act_full": A, "act_rows": A[c * P:(c + 1) * P], "ident": ident}
        for c in range(NCORES)
    ]
    res = run_bass_kernel_spmd(nc, in_maps, list(range(NCORES))).results

    joint_sum = 0.0
    marg_sum = np.zeros(N, dtype=np.float64)
    for c in range(NCORES):
        part = np.asarray(res[c]["partials"], dtype=np.float64)
        joint_sum += part[0]
        marg_sum += part[1:]

    harm = lambda n: sum(1.0 / m for m in range(1, n))
    log_c_d = 0.5 * N * math.log(math.pi) - math.lgamma(N / 2 + 1)
    h_nats = (harm(B) - harm(kk)) + log_c_d + N * 0.5 * joint_sum / B
    joint_bits = h_nats / math.log(2.0)
    marginals = -marg_sum / B
    return np.concatenate([[joint_bits], marginals]).astype(np.float32)
